# revision 16
# baseline (speedup 1.0000x reference)
"""Distributed GraphSAGE (2x SAGEConv + classifier) on 8 TRN2 NeuronCores.

Sharding: destination nodes are sharded 6250/core (the segment_sum outputs);
x is replicated so each core gathers x[src] locally; the small weights are
replicated. Between layers one AllGather shares the hidden states.

Per-core pipeline (per layer):
  - edges (dst in this core's range) sorted by (dst-tile, src-half, src),
    padded per (tile, half) to multiples of 128 with SPMD-shared budgets
  - dma_gather pulls x[src] rows (int16 indices rebased per src-half)
  - one-hot scatter matrices S[e, n] = (dst_rel[e] == n) built on VectorE
  - TensorE: msg[n, f] = sum_blocks S_blk.T @ Xg_blk (PSUM accumulate)
  - mean = msg * 1/max(deg, 1); hT = relu(W_l @ meanT + b + W_r @ xT)
  - layer 1: transpose hT back, DMA to h_local; AllGather -> h_full (bf16)
  - layer 2: classifier out = embT.T @ WcT + bc per tile
"""
import os
import numpy as np

from concourse import bass, bacc, mybir, tile
from concourse.bass_utils import run_bass_kernel_spmd
from concourse.masks import make_identity

# problem constants (hardcoded per harness rules)
N = 50000
E = 640000
D = 128
NCLS = 64
CORES = 8
NSH = N // CORES          # 6250 nodes per core
P = 128
NT = (NSH + P - 1) // P   # 49 node tiles per core
HALF = N // 2             # src-half split for int16 gather indices
CH = int(os.environ.get("GNN_CH", 2048))  # gather chunk slots per dma_gather
SINGLE_PACKET = os.environ.get("GNN_SP", "0") == "1"
PADV = 200.0              # dst_rel pad value (never matches iota 0..127)

last_exec_ns = None


def configure(n, e):
    """Debug hook: shrink the problem (keeps D/NCLS/CORES)."""
    global N, E, NSH, NT, HALF
    N, E = n, e
    NSH = N // CORES
    NT = (NSH + P - 1) // P
    HALF = N // 2


# ----------------------------------------------------------------- host prep
def _prep_core(src, dst, m, budgets=None):
    """Extract + sort core m's edges. Returns per-(tile,half) counts or,
    given shared budgets (blocks per (tile, half)), the packed arrays."""
    sel = (dst >= m * NSH) & (dst < (m + 1) * NSH)
    s = src[sel].astype(np.int64)
    d = (dst[sel] - m * NSH).astype(np.int64)
    t = d >> 7
    half = (s >= HALF).astype(np.int64)
    order = np.lexsort((s, half, t))
    s, d, t, half = s[order], d[order], t[order], half[order]
    cnt = np.zeros((NT, 2), np.int64)
    np.add.at(cnt, (t, half), 1)
    if budgets is None:
        return cnt

    slots = budgets * P                                  # [NT, 2] slots
    base = np.zeros((NT, 2), np.int64)                   # per-stream slot base
    for st in (0, 1):
        base[:, st] = np.concatenate(([0], np.cumsum(slots[:, st])[:-1]))
    # rank of each edge within its (t, half) group
    g = t * 2 + half
    grp_cnt = np.zeros(NT * 2, np.int64)
    np.add.at(grp_cnt, g, 1)
    grp_start = np.concatenate(([0], np.cumsum(grp_cnt)[:-1]))
    rank = np.arange(len(s)) - grp_start[g]
    pos = base[t, half] + rank

    out = {}
    for st in (0, 1):
        L = int(slots[:, st].sum())
        idx = np.zeros(L, np.int64)
        drel = np.full(L, PADV, np.float32)
        msel = half == st
        idx[pos[msel]] = s[msel] - st * HALF
        drel[pos[msel]] = (d[msel] - (t[msel] << 7)).astype(np.float32)
        assert idx.max(initial=0) < 32768
        w16 = idx.astype(np.int16).reshape(L // 16, 16).T
        out[f"idx{st}"] = np.tile(w16, (CORES, 1)).copy()          # [128, L/16]
        out[f"drel{st}"] = drel.reshape(L // P, P).T.copy()        # [128, nblocks]
    deg = np.bincount(d, minlength=NT * P).astype(np.int32)
    out["deg"] = deg.reshape(NT, P).T.copy()                       # [128, NT]
    return out


def _host_prep(x, edge_index, W1l, b1l, W1r, W2l, b2l, W2r, Wc, bc):
    src = np.asarray(edge_index[0], np.int64)
    dst = np.asarray(edge_index[1], np.int64)
    x = np.ascontiguousarray(np.asarray(x, np.float32))

    cnts = np.stack([_prep_core(src, dst, m) for m in range(CORES)])  # [C,NT,2]
    budgets = ((cnts.max(axis=0) + P - 1) // P).astype(np.int64)      # [NT,2]

    iota = np.broadcast_to(np.arange(P, dtype=np.float32), (P, P)).copy()
    common = {
        "x_full": x,
        "iota": iota,
        "w1lt": np.ascontiguousarray(np.asarray(W1l, np.float32).T),
        "w1rt": np.ascontiguousarray(np.asarray(W1r, np.float32).T),
        "w2lt": np.ascontiguousarray(np.asarray(W2l, np.float32).T),
        "w2rt": np.ascontiguousarray(np.asarray(W2r, np.float32).T),
        "wct": np.ascontiguousarray(np.asarray(Wc, np.float32).T),
        "b1l": np.asarray(b1l, np.float32).reshape(D, 1).copy(),
        "b2l": np.asarray(b2l, np.float32).reshape(D, 1).copy(),
        "bcb": np.tile(np.asarray(bc, np.float32), (P, 1)).copy(),
    }
    in_maps = []
    for m in range(CORES):
        core = _prep_core(src, dst, m, budgets)
        xT = np.zeros((D, NT * P), np.float32)
        xT[:, :NSH] = x[m * NSH:(m + 1) * NSH].T
        core["xt"] = xT
        core.update(common)
        in_maps.append(core)
    return in_maps, budgets


# ------------------------------------------------------------- device build
def _build(nc: bacc.Bacc, budgets):
    bf16 = mybir.dt.bfloat16
    f32 = mybir.dt.float32

    slots = budgets * P
    L_st = [int(slots[:, st].sum()) for st in (0, 1)]
    base = np.zeros((NT, 2), np.int64)
    for st in (0, 1):
        base[:, st] = np.concatenate(([0], np.cumsum(slots[:, st])[:-1]))

    # DRAM parameters
    x_full = nc.declare_dram_parameter("x_full", [N, D], f32, isOutput=False)
    xt = nc.declare_dram_parameter("xt", [D, NT * P], f32, isOutput=False)
    iota = nc.declare_dram_parameter("iota", [P, P], f32, isOutput=False)
    idx_p = [nc.declare_dram_parameter(f"idx{st}", [P, L_st[st] // 16],
                                       mybir.dt.int16, isOutput=False)
             for st in (0, 1)]
    drel_p = [nc.declare_dram_parameter(f"drel{st}", [P, L_st[st] // P], f32,
                                        isOutput=False) for st in (0, 1)]
    deg_p = nc.declare_dram_parameter("deg", [P, NT], mybir.dt.int32, isOutput=False)
    w_p = {k: nc.declare_dram_parameter(k, [D, D], f32, isOutput=False)
           for k in ("w1lt", "w1rt", "w2lt", "w2rt")}
    wct_p = nc.declare_dram_parameter("wct", [D, NCLS], f32, isOutput=False)
    b1l_p = nc.declare_dram_parameter("b1l", [D, 1], f32, isOutput=False)
    b2l_p = nc.declare_dram_parameter("b2l", [D, 1], f32, isOutput=False)
    bcb_p = nc.declare_dram_parameter("bcb", [P, NCLS], f32, isOutput=False)
    out_p = nc.declare_dram_parameter("out", [NSH, NCLS], f32, isOutput=True)

    h_local = nc.dram_tensor("h_local", [NSH, D], bf16)
    h_full = nc.dram_tensor("h_full", [N, D], bf16, addr_space="Shared")

    def bcast_mid(ap2d, nb):
        """[128, X] AP -> [128, nb, X] re-reading the X cols nb times."""
        return bass.AP(ap2d.tensor, ap2d.offset, [ap2d.ap[0], [0, nb], list(ap2d.ap[1])])

    def bcast_last(ap2d, n):
        """[128, X] AP -> [128, X, n] broadcasting each element n times."""
        return bass.AP(ap2d.tensor, ap2d.offset, [ap2d.ap[0], list(ap2d.ap[1]), [0, n]])

    with tile.TileContext(nc) as tc:
        with (
            tc.tile_pool(name="cst", bufs=1) as cst,
            tc.tile_pool(name="sb", bufs=2) as sb,
            tc.tile_pool(name="xgp", bufs=2) as xgp,
            tc.tile_pool(name="xbp", bufs=3) as xbp,
            tc.tile_pool(name="sp", bufs=3) as spool,
            tc.tile_pool(name="ps", bufs=2, space="PSUM") as ps,
        ):
            # ---- constants ----
            iota_sb = cst.tile([P, P], f32)
            nc.scalar.dma_start(out=iota_sb[:, :], in_=iota[:, :])
            ident = cst.tile([P, P], bf16)
            make_identity(nc, ident[:, :])

            wb = {}
            for k in ("w1lt", "w1rt", "w2lt", "w2rt"):
                wf = sb.tile([D, D], f32, tag="wload")
                nc.scalar.dma_start(out=wf[:, :], in_=w_p[k][:, :])
                wb[k] = cst.tile([D, D], bf16, tag=f"w_{k}", name=f"w_{k}")
                nc.vector.tensor_copy(wb[k][:, :], wf[:, :])
            wcf = sb.tile([D, NCLS], f32, tag="wload")
            nc.scalar.dma_start(out=wcf[:, :], in_=wct_p[:, :])
            wcb = cst.tile([D, NCLS], bf16)
            nc.vector.tensor_copy(wcb[:, :], wcf[:, :])

            b1l_sb = cst.tile([D, 1], f32)
            nc.scalar.dma_start(out=b1l_sb[:, :], in_=b1l_p[:, :])
            b2l_sb = cst.tile([D, 1], f32)
            nc.scalar.dma_start(out=b2l_sb[:, :], in_=b2l_p[:, :])
            bcb_sb = cst.tile([P, NCLS], f32)
            nc.scalar.dma_start(out=bcb_sb[:, :], in_=bcb_p[:, :])

            # inverse degree: 1 / max(deg, 1)
            deg_i = sb.tile([P, NT], mybir.dt.int32, tag="degl")
            nc.scalar.dma_start(out=deg_i[:, :], in_=deg_p[:, :])
            deg_f = sb.tile([P, NT], f32, tag="degf")
            nc.vector.tensor_copy(deg_f[:, :], deg_i[:, :])
            nc.vector.tensor_scalar_max(deg_f[:, :], deg_f[:, :], 1.0)
            invdeg = cst.tile([P, NT], f32)
            nc.vector.reciprocal(invdeg[:, :], deg_f[:, :])

            # xT in bf16 (padded feature-major local x)
            xt_f = cst.tile([D, NT * P], f32)
            nc.scalar.dma_start(out=xt_f[:, :], in_=xt[:, :])
            xt_b = cst.tile([D, NT * P], bf16)
            nc.vector.tensor_copy(xt_b[:, :], xt_f[:, :])

            # hT persistent (layer-1 output, feature-major, bf16)
            ht_b = cst.tile([D, NT * P], bf16)

            # gather metadata in SBUF
            idx_sb, drel_sb = [], []
            for st in (0, 1):
                it = cst.tile([P, L_st[st] // 16], mybir.dt.int16, tag=f"idxsb{st}", name=f"idxsb{st}")
                nc.scalar.dma_start(out=it[:, :], in_=idx_p[st][:, :])
                idx_sb.append(it)
                dt_ = cst.tile([P, L_st[st] // P], f32, tag=f"drelsb{st}", name=f"drelsb{st}")
                nc.scalar.dma_start(out=dt_[:, :], in_=drel_p[st][:, :])
                drel_sb.append(dt_)

            # ------------------------------------------------ one layer
            def do_layer(lay):
                if "nooff" in os.environ.get("GNN_DEBUG", ""):
                    src_half = [x_full[0:HALF, :], x_full[0:HALF, :]]
                elif lay == 0:
                    src_half = [x_full[0:HALF, :], x_full[HALF:N, :]]
                else:
                    src_half = [h_full[0:HALF, :], h_full[HALF:N, :]]
                wl = wb["w1lt"] if lay == 0 else wb["w2lt"]
                wr = wb["w1rt"] if lay == 0 else wb["w2rt"]
                bias = b1l_sb if lay == 0 else b2l_sb
                rhs_loc = xt_b if lay == 0 else ht_b

                chunk_tiles = [{}, {}]  # per stream: chunk id -> bf16 tile

                dbg = os.environ.get("GNN_DEBUG", "")

                def get_chunk(st, c):
                    if c in chunk_tiles[st]:
                        return chunk_tiles[st][c]
                    ln = min(CH, L_st[st] - c * CH)
                    idx_ap = idx_sb[st][:, c * CH // 16:(c * CH + ln) // 16]
                    if "nogather" in dbg:
                        xb = xbp.tile([P, CH // P, D], bf16, tag=f"xb{st}")
                        nc.vector.memset(xb[:, :, :], 1.0)
                    elif lay == 0:
                        xg = xgp.tile([P, CH // P, D], f32, tag=f"xg{st}")
                        nc.gpsimd.dma_gather(
                            out_ap=xg[:, :ln // P, :], in_ap=src_half[st],
                            idxs_ap=idx_ap, num_idxs=ln, num_idxs_reg=ln,
                            elem_size=D, single_packet=SINGLE_PACKET)
                        xb = xbp.tile([P, CH // P, D], bf16, tag=f"xb{st}")
                        nc.scalar.activation(xb[:, :ln // P, :], xg[:, :ln // P, :],
                                             mybir.ActivationFunctionType.Copy)
                    else:
                        xb = xbp.tile([P, CH // P, D], bf16, tag=f"xb{st}")
                        nc.gpsimd.dma_gather(
                            out_ap=xb[:, :ln // P, :], in_ap=src_half[st],
                            idxs_ap=idx_ap, num_idxs=ln, num_idxs_reg=ln,
                            elem_size=D, single_packet=SINGLE_PACKET)
                    chunk_tiles[st][c] = xb
                    return xb

                for t in range(NT):
                    rows = min(P, NSH - t * P)
                    pm = ps.tile([P, D], f32, tag="msg")
                    blocks = []
                    for st in (0, 1):
                        nb = int(budgets[t, st])
                        if nb == 0:
                            continue
                        b0 = int(base[t, st]) // P
                        # S for this (tile, stream): [128, nb, 128] bf16
                        S = spool.tile([P, nb, P], bf16, tag="s")
                        d_ap = drel_sb[st][:, b0:b0 + nb]
                        if "nos" in dbg:
                            nc.vector.memset(S[:, :, :], 0.0)
                        else:
                            nc.vector.tensor_tensor(
                                out=S[:, :, :], in0=bcast_mid(iota_sb[:, :], nb),
                                in1=bcast_last(d_ap, P), op=mybir.AluOpType.is_equal)
                        for b in range(nb):
                            slot = int(base[t, st]) + b * P
                            xb = get_chunk(st, slot // CH)
                            blocks.append((S[:, b, :], xb[:, (slot % CH) // P, :]))
                    for i, (s_ap, x_ap) in enumerate(blocks):
                        nc.tensor.matmul(pm[:, :], lhsT=s_ap, rhs=x_ap,
                                         start=(i == 0), stop=(i == len(blocks) - 1))

                    # mean (node-major, bf16) then transpose to feature-major
                    mean_b = sb.tile([P, D], bf16, tag="mean")
                    nc.vector.tensor_scalar(
                        out=mean_b[:, :], in0=pm[:, :],
                        scalar1=invdeg[:, t:t + 1], scalar2=None,
                        op0=mybir.AluOpType.mult)
                    meanT = sb.tile([P, D], bf16, tag="meanT")
                    if "notr" in dbg:
                        nc.vector.tensor_copy(meanT[:, :], mean_b[:, :])
                    else:
                        pt = ps.tile([P, D], bf16, tag="tr")
                        nc.tensor.transpose(pt[:, :], mean_b[:, :], ident[:, :])
                        nc.scalar.activation(meanT[:, :], pt[:, :],
                                             mybir.ActivationFunctionType.Copy)

                    # hT[o, n] = relu(Wl @ meanT + Wr @ xT + b)
                    ph = ps.tile([D, P], f32, tag="hT")
                    nc.tensor.matmul(ph[:, :], lhsT=wl[:, :], rhs=meanT[:, :],
                                     start=True, stop=False)
                    nc.tensor.matmul(ph[:, :], lhsT=wr[:, :],
                                     rhs=rhs_loc[:, t * P:(t + 1) * P],
                                     start=False, stop=True)

                    if lay == 0:
                        hT = ht_b[:, t * P:(t + 1) * P]
                        if "norelu" in dbg:
                            nc.scalar.activation(hT, ph[:, :],
                                                 mybir.ActivationFunctionType.Copy)
                        else:
                            nc.scalar.activation(hT, ph[:, :],
                                                 mybir.ActivationFunctionType.Relu,
                                                 bias=bias[:, :])
                        h_sb = sb.tile([P, D], bf16, tag="hs")
                        if "notr" in dbg:
                            nc.vector.tensor_copy(h_sb[:, :], hT)
                        else:
                            # node-major copy for the AllGather / layer-2 gather
                            phn = ps.tile([P, D], bf16, tag="aux")
                            nc.tensor.transpose(phn[:, :], hT, ident[:, :])
                            nc.vector.tensor_copy(h_sb[:, :], phn[:, :])
                        nc.scalar.dma_start(out=h_local[t * P:t * P + rows, :],
                                          in_=h_sb[:rows, :])
                    else:
                        embT = sb.tile([D, P], bf16, tag="embT")
                        nc.scalar.activation(embT[:, :], ph[:, :],
                                             mybir.ActivationFunctionType.Relu,
                                             bias=bias[:, :])
                        pc = ps.tile([P, NCLS], f32, tag="aux")
                        nc.tensor.matmul(pc[:, :], lhsT=embT[:, :], rhs=wcb[:, :],
                                         start=True, stop=True)
                        oc = sb.tile([P, NCLS], f32, tag="oc")
                        nc.vector.tensor_tensor(out=oc[:, :], in0=pc[:, :],
                                                in1=bcb_sb[:, :],
                                                op=mybir.AluOpType.add)
                        nc.scalar.dma_start(out=out_p[t * P:t * P + rows, :],
                                          in_=oc[:rows, :])

            dbg = os.environ.get("GNN_DEBUG", "")
            do_layer(0)
            if "nocoll" in dbg:
                nc.gpsimd.dma_start(out=h_full[0:NSH, :], in_=h_local[:, :])
            else:
                nc.gpsimd.collective_compute(
                    "AllGather", mybir.AluOpType.bypass,
                    replica_groups=[list(range(CORES))],
                    ins=[h_local[:, :].opt()], outs=[h_full[:, :].opt()])
            if "nolay2" not in dbg:
                do_layer(1)
            else:
                for t in range(NT):
                    rows = min(P, NSH - t * P)
                    oc = sb.tile([P, NCLS], f32, tag="oc")
                    nc.vector.tensor_copy(oc[:, :], bcb_sb[:, :])
                    nc.scalar.dma_start(out=out_p[t * P:t * P + rows, :],
                                        in_=oc[:rows, :])
    return nc


# ------------------------------------------------------------------- driver
def _enable_axon_trace():
    """The agent image's antenv lacks axon_hooks; synthesize it from the
    ctypes NTFF hook in trn_agent_boot so trace=True works under axon."""
    import sys
    import types
    try:
        import antenv.axon_hooks  # noqa: F401
        return True
    except ImportError:
        pass
    try:
        from trn_agent_boot.trn_boot import _ntff_profile_via_ctypes
        hook = _ntff_profile_via_ctypes("/opt/axon/libaxon_pjrt.so")
        if hook is None:
            return False
        mod = types.ModuleType("antenv.axon_hooks")
        mod.get_axon_ntff_profile_hook = lambda: hook
        mod.set_axon_ntff_profile_hook = lambda h: None
        sys.modules["antenv.axon_hooks"] = mod
        # artifact upload needs bucket access we don't have here
        from concourse import bass_utils as _bu
        _bu.upload_artifacts = lambda tmpdir: f"file://{tmpdir}"
        return True
    except Exception:
        return False


def kernel(x, edge_index, W1l, b1l, W1r, W2l, b2l, W2r, Wc, bc):
    global last_exec_ns
    in_maps, budgets = _host_prep(x, edge_index, W1l, b1l, W1r, W2l, b2l, W2r,
                                  Wc, bc)
    nc = _build(bacc.Bacc(), budgets)
    nc.compile()
    trace = os.environ.get("GNN_TRACE", "0") == "1" and _enable_axon_trace()
    r = run_bass_kernel_spmd(nc, in_maps, core_ids=list(range(CORES)),
                             trace=trace)
    last_exec_ns = r.exec_time_ns
    out = np.concatenate([r.results[m]["out"] for m in range(CORES)], axis=0)
    return out.astype(np.float32)


# revision 18
# speedup vs baseline: 1.6798x; 1.6798x over previous
"""Distributed GraphSAGE (2x SAGEConv + classifier) on 8 TRN2 NeuronCores.

Sharding: destination nodes are sharded 6250/core (the segment_sum outputs);
x is replicated so each core gathers x[src] locally; the small weights are
replicated. Between layers one AllGather shares the hidden states.

Per-core pipeline (per layer):
  - edges (dst in this core's range) sorted by (dst-tile, src-half, src),
    padded per (tile, half) to multiples of 128 with SPMD-shared budgets
  - dma_gather pulls x[src] rows (int16 indices rebased per src-half)
  - one-hot scatter matrices S[e, n] = (dst_rel[e] == n) built on VectorE
  - TensorE: msg[n, f] = sum_blocks S_blk.T @ Xg_blk (PSUM accumulate)
  - mean = msg * 1/max(deg, 1); hT = relu(W_l @ meanT + b + W_r @ xT)
  - layer 1: transpose hT back, DMA to h_local; AllGather -> h_full (bf16)
  - layer 2: classifier out = embT.T @ WcT + bc per tile
"""
import os
import numpy as np

from concourse import bass, bacc, mybir, tile
from concourse.bass_utils import run_bass_kernel_spmd
from concourse.masks import make_identity

# problem constants (hardcoded per harness rules)
N = 50000
E = 640000
D = 128
NCLS = 64
CORES = 8
NSH = N // CORES          # 6250 nodes per core
P = 128
NT = (NSH + P - 1) // P   # 49 node tiles per core
HALF = N // 2             # src-half split for int16 gather indices
CH = int(os.environ.get("GNN_CH", 2048))  # gather chunk slots per dma_gather
SINGLE_PACKET = os.environ.get("GNN_SP", "0") == "1"
NQ = int(os.environ.get("GNN_NQ", 4))  # swdge queues used for gather DGE
PADV = 200.0              # dst_rel pad value (never matches iota 0..127)

last_exec_ns = None


def configure(n, e):
    """Debug hook: shrink the problem (keeps D/NCLS/CORES)."""
    global N, E, NSH, NT, HALF
    N, E = n, e
    NSH = N // CORES
    NT = (NSH + P - 1) // P
    HALF = N // 2


# ----------------------------------------------------------------- host prep
def _prep_core(src, dst, m, budgets=None):
    """Extract + sort core m's edges. Returns per-(tile,half) counts or,
    given shared budgets (blocks per (tile, half)), the packed arrays."""
    sel = (dst >= m * NSH) & (dst < (m + 1) * NSH)
    s = src[sel].astype(np.int64)
    d = (dst[sel] - m * NSH).astype(np.int64)
    t = d >> 7
    half = (s >= HALF).astype(np.int64)
    order = np.lexsort((s, half, t))
    s, d, t, half = s[order], d[order], t[order], half[order]
    cnt = np.zeros((NT, 2), np.int64)
    np.add.at(cnt, (t, half), 1)
    if budgets is None:
        return cnt

    slots = budgets * P                                  # [NT, 2] slots
    base = np.zeros((NT, 2), np.int64)                   # per-stream slot base
    for st in (0, 1):
        base[:, st] = np.concatenate(([0], np.cumsum(slots[:, st])[:-1]))
    # rank of each edge within its (t, half) group
    g = t * 2 + half
    grp_cnt = np.zeros(NT * 2, np.int64)
    np.add.at(grp_cnt, g, 1)
    grp_start = np.concatenate(([0], np.cumsum(grp_cnt)[:-1]))
    rank = np.arange(len(s)) - grp_start[g]
    pos = base[t, half] + rank

    out = {}
    for st in (0, 1):
        L = int(slots[:, st].sum())
        idx = np.zeros(L, np.int64)
        drel = np.full(L, PADV, np.float32)
        msel = half == st
        idx[pos[msel]] = s[msel] - st * HALF
        drel[pos[msel]] = (d[msel] - (t[msel] << 7)).astype(np.float32)
        assert idx.max(initial=0) < 32768
        w16 = idx.astype(np.int16).reshape(L // 16, 16).T
        out[f"idx{st}"] = np.tile(w16, (CORES, 1)).copy()          # [128, L/16]
        out[f"drel{st}"] = drel.reshape(L // P, P).T.copy()        # [128, nblocks]
    deg = np.bincount(d, minlength=NT * P).astype(np.int32)
    out["deg"] = deg.reshape(NT, P).T.copy()                       # [128, NT]
    return out


def _host_prep(x, edge_index, W1l, b1l, W1r, W2l, b2l, W2r, Wc, bc):
    src = np.asarray(edge_index[0], np.int64)
    dst = np.asarray(edge_index[1], np.int64)
    x = np.ascontiguousarray(np.asarray(x, np.float32))

    cnts = np.stack([_prep_core(src, dst, m) for m in range(CORES)])  # [C,NT,2]
    budgets = ((cnts.max(axis=0) + P - 1) // P).astype(np.int64)      # [NT,2]

    iota = np.broadcast_to(np.arange(P, dtype=np.float32), (P, P)).copy()
    common = {
        "x_full": x,
        "iota": iota,
        "w1lt": np.ascontiguousarray(np.asarray(W1l, np.float32).T),
        "w1rt": np.ascontiguousarray(np.asarray(W1r, np.float32).T),
        "w2lt": np.ascontiguousarray(np.asarray(W2l, np.float32).T),
        "w2rt": np.ascontiguousarray(np.asarray(W2r, np.float32).T),
        "wct": np.ascontiguousarray(np.asarray(Wc, np.float32).T),
        "b1l": np.asarray(b1l, np.float32).reshape(D, 1).copy(),
        "b2l": np.asarray(b2l, np.float32).reshape(D, 1).copy(),
        "bcb": np.tile(np.asarray(bc, np.float32), (P, 1)).copy(),
    }
    in_maps = []
    for m in range(CORES):
        core = _prep_core(src, dst, m, budgets)
        xT = np.zeros((D, NT * P), np.float32)
        xT[:, :NSH] = x[m * NSH:(m + 1) * NSH].T
        core["xt"] = xT
        core.update(common)
        in_maps.append(core)
    return in_maps, budgets


# ------------------------------------------------------------- device build
def _build(nc: bacc.Bacc, budgets):
    bf16 = mybir.dt.bfloat16
    f32 = mybir.dt.float32

    slots = budgets * P
    L_st = [int(slots[:, st].sum()) for st in (0, 1)]
    base = np.zeros((NT, 2), np.int64)
    for st in (0, 1):
        base[:, st] = np.concatenate(([0], np.cumsum(slots[:, st])[:-1]))

    # DRAM parameters
    x_full = nc.declare_dram_parameter("x_full", [N, D], f32, isOutput=False)
    xt = nc.declare_dram_parameter("xt", [D, NT * P], f32, isOutput=False)
    iota = nc.declare_dram_parameter("iota", [P, P], f32, isOutput=False)
    idx_p = [nc.declare_dram_parameter(f"idx{st}", [P, L_st[st] // 16],
                                       mybir.dt.int16, isOutput=False)
             for st in (0, 1)]
    drel_p = [nc.declare_dram_parameter(f"drel{st}", [P, L_st[st] // P], f32,
                                        isOutput=False) for st in (0, 1)]
    deg_p = nc.declare_dram_parameter("deg", [P, NT], mybir.dt.int32, isOutput=False)
    w_p = {k: nc.declare_dram_parameter(k, [D, D], f32, isOutput=False)
           for k in ("w1lt", "w1rt", "w2lt", "w2rt")}
    wct_p = nc.declare_dram_parameter("wct", [D, NCLS], f32, isOutput=False)
    b1l_p = nc.declare_dram_parameter("b1l", [D, 1], f32, isOutput=False)
    b2l_p = nc.declare_dram_parameter("b2l", [D, 1], f32, isOutput=False)
    bcb_p = nc.declare_dram_parameter("bcb", [P, NCLS], f32, isOutput=False)
    out_p = nc.declare_dram_parameter("out", [NSH, NCLS], f32, isOutput=True)

    h_local = nc.dram_tensor("h_local", [NSH, D], bf16)
    h_full = nc.dram_tensor("h_full", [N, D], bf16, addr_space="Shared")

    def bcast_mid(ap2d, nb):
        """[128, X] AP -> [128, nb, X] re-reading the X cols nb times."""
        return bass.AP(ap2d.tensor, ap2d.offset, [ap2d.ap[0], [0, nb], list(ap2d.ap[1])])

    def bcast_last(ap2d, n):
        """[128, X] AP -> [128, X, n] broadcasting each element n times."""
        return bass.AP(ap2d.tensor, ap2d.offset, [ap2d.ap[0], list(ap2d.ap[1]), [0, n]])

    with tile.TileContext(nc) as tc:
        with (
            tc.tile_pool(name="cst", bufs=1) as cst,
            tc.tile_pool(name="sb", bufs=2) as sb,
            tc.tile_pool(name="xgp", bufs=2) as xgp,
            tc.tile_pool(name="xbp", bufs=3) as xbp,
            tc.tile_pool(name="sp", bufs=3) as spool,
            tc.tile_pool(name="ps", bufs=2, space="PSUM") as ps,
        ):
            # ---- constants ----
            iota_sb = cst.tile([P, P], f32)
            nc.scalar.dma_start(out=iota_sb[:, :], in_=iota[:, :])
            ident = cst.tile([P, P], bf16)
            make_identity(nc, ident[:, :])

            wb = {}
            for k in ("w1lt", "w1rt", "w2lt", "w2rt"):
                wf = sb.tile([D, D], f32, tag="wload")
                nc.scalar.dma_start(out=wf[:, :], in_=w_p[k][:, :])
                wb[k] = cst.tile([D, D], bf16, tag=f"w_{k}", name=f"w_{k}")
                nc.vector.tensor_copy(wb[k][:, :], wf[:, :])
            wcf = sb.tile([D, NCLS], f32, tag="wload")
            nc.scalar.dma_start(out=wcf[:, :], in_=wct_p[:, :])
            wcb = cst.tile([D, NCLS], bf16)
            nc.vector.tensor_copy(wcb[:, :], wcf[:, :])

            b1l_sb = cst.tile([D, 1], f32)
            nc.scalar.dma_start(out=b1l_sb[:, :], in_=b1l_p[:, :])
            b2l_sb = cst.tile([D, 1], f32)
            nc.scalar.dma_start(out=b2l_sb[:, :], in_=b2l_p[:, :])
            bcb_sb = cst.tile([P, NCLS], f32)
            nc.scalar.dma_start(out=bcb_sb[:, :], in_=bcb_p[:, :])

            # inverse degree: 1 / max(deg, 1)
            deg_i = sb.tile([P, NT], mybir.dt.int32, tag="degl")
            nc.scalar.dma_start(out=deg_i[:, :], in_=deg_p[:, :])
            deg_f = sb.tile([P, NT], f32, tag="degf")
            nc.vector.tensor_copy(deg_f[:, :], deg_i[:, :])
            nc.vector.tensor_scalar_max(deg_f[:, :], deg_f[:, :], 1.0)
            invdeg = cst.tile([P, NT], f32)
            nc.vector.reciprocal(invdeg[:, :], deg_f[:, :])

            # xT in bf16 (padded feature-major local x)
            xt_f = cst.tile([D, NT * P], f32)
            nc.scalar.dma_start(out=xt_f[:, :], in_=xt[:, :])
            xt_b = cst.tile([D, NT * P], bf16)
            nc.vector.tensor_copy(xt_b[:, :], xt_f[:, :])

            # hT persistent (layer-1 output, feature-major, bf16)
            ht_b = cst.tile([D, NT * P], bf16)

            # gather metadata in SBUF
            idx_sb, drel_sb = [], []
            for st in (0, 1):
                it = cst.tile([P, L_st[st] // 16], mybir.dt.int16, tag=f"idxsb{st}", name=f"idxsb{st}")
                nc.scalar.dma_start(out=it[:, :], in_=idx_p[st][:, :])
                idx_sb.append(it)
                dt_ = cst.tile([P, L_st[st] // P], f32, tag=f"drelsb{st}", name=f"drelsb{st}")
                nc.scalar.dma_start(out=dt_[:, :], in_=drel_p[st][:, :])
                drel_sb.append(dt_)

            # ------------------------------------------------ one layer
            def do_layer(lay):
                if "nooff" in os.environ.get("GNN_DEBUG", ""):
                    src_half = [x_full[0:HALF, :], x_full[0:HALF, :]]
                elif lay == 0:
                    src_half = [x_full[0:HALF, :], x_full[HALF:N, :]]
                else:
                    src_half = [h_full[0:HALF, :], h_full[HALF:N, :]]
                wl = wb["w1lt"] if lay == 0 else wb["w2lt"]
                wr = wb["w1rt"] if lay == 0 else wb["w2rt"]
                bias = b1l_sb if lay == 0 else b2l_sb
                rhs_loc = xt_b if lay == 0 else ht_b

                chunk_tiles = [{}, {}]  # per stream: chunk id -> bf16 tile
                qrr = [0]  # round-robin swdge queue for gather DGE parallelism

                dbg = os.environ.get("GNN_DEBUG", "")

                def get_chunk(st, c):
                    if c in chunk_tiles[st]:
                        return chunk_tiles[st][c]
                    ln = min(CH, L_st[st] - c * CH)
                    idx_ap = idx_sb[st][:, c * CH // 16:(c * CH + ln) // 16]
                    if "nogather" in dbg:
                        xb = xbp.tile([P, CH // P, D], bf16, tag=f"xb{st}")
                        nc.vector.memset(xb[:, :, :], 1.0)
                    elif lay == 0:
                        xg = xgp.tile([P, CH // P, D], f32, tag=f"xg{st}")
                        nc.gpsimd.dma_gather(
                            out_ap=xg[:, :ln // P, :], in_ap=src_half[st],
                            idxs_ap=idx_ap, num_idxs=ln, num_idxs_reg=ln,
                            elem_size=D, single_packet=SINGLE_PACKET,
                            queue_num=qrr[0])
                        qrr[0] = (qrr[0] + 1) % NQ
                        xb = xbp.tile([P, CH // P, D], bf16, tag=f"xb{st}")
                        nc.scalar.activation(xb[:, :ln // P, :], xg[:, :ln // P, :],
                                             mybir.ActivationFunctionType.Copy)
                    else:
                        xb = xbp.tile([P, CH // P, D], bf16, tag=f"xb{st}")
                        nc.gpsimd.dma_gather(
                            out_ap=xb[:, :ln // P, :], in_ap=src_half[st],
                            idxs_ap=idx_ap, num_idxs=ln, num_idxs_reg=ln,
                            elem_size=D, single_packet=SINGLE_PACKET,
                            queue_num=qrr[0])
                        qrr[0] = (qrr[0] + 1) % NQ
                    chunk_tiles[st][c] = xb
                    return xb

                for t in range(NT):
                    rows = min(P, NSH - t * P)
                    pm = ps.tile([P, D], f32, tag="msg")
                    blocks = []
                    for st in (0, 1):
                        nb = int(budgets[t, st])
                        if nb == 0:
                            continue
                        b0 = int(base[t, st]) // P
                        # S for this (tile, stream): [128, nb, 128] bf16
                        S = spool.tile([P, nb, P], bf16, tag="s")
                        d_ap = drel_sb[st][:, b0:b0 + nb]
                        if "nos" in dbg:
                            nc.vector.memset(S[:, :, :], 0.0)
                        else:
                            nc.vector.tensor_tensor(
                                out=S[:, :, :], in0=bcast_mid(iota_sb[:, :], nb),
                                in1=bcast_last(d_ap, P), op=mybir.AluOpType.is_equal)
                        for b in range(nb):
                            slot = int(base[t, st]) + b * P
                            xb = get_chunk(st, slot // CH)
                            blocks.append((S[:, b, :], xb[:, (slot % CH) // P, :]))
                    for i, (s_ap, x_ap) in enumerate(blocks):
                        nc.tensor.matmul(pm[:, :], lhsT=s_ap, rhs=x_ap,
                                         start=(i == 0), stop=(i == len(blocks) - 1))

                    # mean (node-major, bf16) then transpose to feature-major
                    mean_b = sb.tile([P, D], bf16, tag="mean")
                    nc.vector.tensor_scalar(
                        out=mean_b[:, :], in0=pm[:, :],
                        scalar1=invdeg[:, t:t + 1], scalar2=None,
                        op0=mybir.AluOpType.mult)
                    meanT = sb.tile([P, D], bf16, tag="meanT")
                    if "notr" in dbg:
                        nc.vector.tensor_copy(meanT[:, :], mean_b[:, :])
                    else:
                        pt = ps.tile([P, D], bf16, tag="tr")
                        nc.tensor.transpose(pt[:, :], mean_b[:, :], ident[:, :])
                        nc.scalar.activation(meanT[:, :], pt[:, :],
                                             mybir.ActivationFunctionType.Copy)

                    # hT[o, n] = relu(Wl @ meanT + Wr @ xT + b)
                    ph = ps.tile([D, P], f32, tag="hT")
                    nc.tensor.matmul(ph[:, :], lhsT=wl[:, :], rhs=meanT[:, :],
                                     start=True, stop=False)
                    nc.tensor.matmul(ph[:, :], lhsT=wr[:, :],
                                     rhs=rhs_loc[:, t * P:(t + 1) * P],
                                     start=False, stop=True)

                    if lay == 0:
                        hT = ht_b[:, t * P:(t + 1) * P]
                        if "norelu" in dbg:
                            nc.scalar.activation(hT, ph[:, :],
                                                 mybir.ActivationFunctionType.Copy)
                        else:
                            nc.scalar.activation(hT, ph[:, :],
                                                 mybir.ActivationFunctionType.Relu,
                                                 bias=bias[:, :])
                        h_sb = sb.tile([P, D], bf16, tag="hs")
                        if "notr" in dbg:
                            nc.vector.tensor_copy(h_sb[:, :], hT)
                        else:
                            # node-major copy for the AllGather / layer-2 gather
                            phn = ps.tile([P, D], bf16, tag="aux")
                            nc.tensor.transpose(phn[:, :], hT, ident[:, :])
                            nc.vector.tensor_copy(h_sb[:, :], phn[:, :])
                        nc.scalar.dma_start(out=h_local[t * P:t * P + rows, :],
                                          in_=h_sb[:rows, :])
                    else:
                        embT = sb.tile([D, P], bf16, tag="embT")
                        nc.scalar.activation(embT[:, :], ph[:, :],
                                             mybir.ActivationFunctionType.Relu,
                                             bias=bias[:, :])
                        pc = ps.tile([P, NCLS], f32, tag="aux")
                        nc.tensor.matmul(pc[:, :], lhsT=embT[:, :], rhs=wcb[:, :],
                                         start=True, stop=True)
                        oc = sb.tile([P, NCLS], f32, tag="oc")
                        nc.vector.tensor_tensor(out=oc[:, :], in0=pc[:, :],
                                                in1=bcb_sb[:, :],
                                                op=mybir.AluOpType.add)
                        nc.scalar.dma_start(out=out_p[t * P:t * P + rows, :],
                                          in_=oc[:rows, :])

            dbg = os.environ.get("GNN_DEBUG", "")
            do_layer(0)
            if "nocoll" in dbg:
                nc.gpsimd.dma_start(out=h_full[0:NSH, :], in_=h_local[:, :])
            else:
                nc.gpsimd.collective_compute(
                    "AllGather", mybir.AluOpType.bypass,
                    replica_groups=[list(range(CORES))],
                    ins=[h_local[:, :].opt()], outs=[h_full[:, :].opt()])
            if "nolay2" not in dbg:
                do_layer(1)
            else:
                for t in range(NT):
                    rows = min(P, NSH - t * P)
                    oc = sb.tile([P, NCLS], f32, tag="oc")
                    nc.vector.tensor_copy(oc[:, :], bcb_sb[:, :])
                    nc.scalar.dma_start(out=out_p[t * P:t * P + rows, :],
                                        in_=oc[:rows, :])
    return nc


# ------------------------------------------------------------------- driver
def _enable_axon_trace():
    """The agent image's antenv lacks axon_hooks; synthesize it from the
    ctypes NTFF hook in trn_agent_boot so trace=True works under axon."""
    import sys
    import types
    try:
        import antenv.axon_hooks  # noqa: F401
        return True
    except ImportError:
        pass
    try:
        from trn_agent_boot.trn_boot import _ntff_profile_via_ctypes
        hook = _ntff_profile_via_ctypes("/opt/axon/libaxon_pjrt.so")
        if hook is None:
            return False
        mod = types.ModuleType("antenv.axon_hooks")
        mod.get_axon_ntff_profile_hook = lambda: hook
        mod.set_axon_ntff_profile_hook = lambda h: None
        sys.modules["antenv.axon_hooks"] = mod
        # artifact upload needs bucket access we don't have here
        from concourse import bass_utils as _bu
        _bu.upload_artifacts = lambda tmpdir: f"file://{tmpdir}"
        return True
    except Exception:
        return False


def kernel(x, edge_index, W1l, b1l, W1r, W2l, b2l, W2r, Wc, bc):
    global last_exec_ns
    in_maps, budgets = _host_prep(x, edge_index, W1l, b1l, W1r, W2l, b2l, W2r,
                                  Wc, bc)
    nc = _build(bacc.Bacc(num_swdge_queues=NQ), budgets)
    nc.compile()
    trace = os.environ.get("GNN_TRACE", "0") == "1" and _enable_axon_trace()
    r = run_bass_kernel_spmd(nc, in_maps, core_ids=list(range(CORES)),
                             trace=trace)
    last_exec_ns = r.exec_time_ns
    out = np.concatenate([r.results[m]["out"] for m in range(CORES)], axis=0)
    return out.astype(np.float32)


# revision 19
# speedup vs baseline: 2.1217x; 1.2631x over previous
"""Distributed GraphSAGE (2x SAGEConv + classifier) on 8 TRN2 NeuronCores.

Sharding: destination nodes are sharded 6250/core (the segment_sum outputs);
x is replicated so each core gathers x[src] locally; the small weights are
replicated. Between layers one AllGather shares the hidden states.

Per-core pipeline (per layer):
  - edges (dst in this core's range) sorted by (dst-tile, src-half, src),
    padded per (tile, half) to multiples of 128 with SPMD-shared budgets
  - dma_gather pulls x[src] rows (int16 indices rebased per src-half)
  - one-hot scatter matrices S[e, n] = (dst_rel[e] == n) built on VectorE
  - TensorE: msg[n, f] = sum_blocks S_blk.T @ Xg_blk (PSUM accumulate)
  - mean = msg * 1/max(deg, 1); hT = relu(W_l @ meanT + b + W_r @ xT)
  - layer 1: transpose hT back, DMA to h_local; AllGather -> h_full (bf16)
  - layer 2: classifier out = embT.T @ WcT + bc per tile
"""
import os
import ml_dtypes
import numpy as np

from concourse import bass, bacc, mybir, tile
from concourse.bass_utils import run_bass_kernel_spmd
from concourse.masks import make_identity

# problem constants (hardcoded per harness rules)
N = 50000
E = 640000
D = 128
NCLS = 64
CORES = 8
NSH = N // CORES          # 6250 nodes per core
P = 128
NT = (NSH + P - 1) // P   # 49 node tiles per core
HALF = N // 2             # src-half split for int16 gather indices
CH = int(os.environ.get("GNN_CH", 1024))  # gather chunk slots per dma_gather
SINGLE_PACKET = os.environ.get("GNN_SP", "0") == "1"
NQ = int(os.environ.get("GNN_NQ", 4))  # swdge queues used for gather DGE
PADV = 200.0              # dst_rel pad value (never matches iota 0..127)

last_exec_ns = None


def configure(n, e):
    """Debug hook: shrink the problem (keeps D/NCLS/CORES)."""
    global N, E, NSH, NT, HALF
    N, E = n, e
    NSH = N // CORES
    NT = (NSH + P - 1) // P
    HALF = N // 2


# ----------------------------------------------------------------- host prep
def _prep_core(src, dst, m, budgets=None):
    """Extract + sort core m's edges. Returns per-(tile,half) counts or,
    given shared budgets (blocks per (tile, half)), the packed arrays."""
    sel = (dst >= m * NSH) & (dst < (m + 1) * NSH)
    s = src[sel].astype(np.int64)
    d = (dst[sel] - m * NSH).astype(np.int64)
    t = d >> 7
    half = (s >= HALF).astype(np.int64)
    order = np.lexsort((s, half, t))
    s, d, t, half = s[order], d[order], t[order], half[order]
    cnt = np.zeros((NT, 2), np.int64)
    np.add.at(cnt, (t, half), 1)
    if budgets is None:
        return cnt

    slots = budgets * P                                  # [NT, 2] slots
    base = np.zeros((NT, 2), np.int64)                   # per-stream slot base
    for st in (0, 1):
        base[:, st] = np.concatenate(([0], np.cumsum(slots[:, st])[:-1]))
    # rank of each edge within its (t, half) group
    g = t * 2 + half
    grp_cnt = np.zeros(NT * 2, np.int64)
    np.add.at(grp_cnt, g, 1)
    grp_start = np.concatenate(([0], np.cumsum(grp_cnt)[:-1]))
    rank = np.arange(len(s)) - grp_start[g]
    pos = base[t, half] + rank

    out = {}
    for st in (0, 1):
        L = int(slots[:, st].sum())
        idx = np.zeros(L, np.int64)
        drel = np.full(L, PADV, np.float32)
        msel = half == st
        idx[pos[msel]] = s[msel] - st * HALF
        drel[pos[msel]] = (d[msel] - (t[msel] << 7)).astype(np.float32)
        assert idx.max(initial=0) < 32768
        w16 = idx.astype(np.int16).reshape(L // 16, 16).T
        out[f"idx{st}"] = np.tile(w16, (CORES, 1)).copy()          # [128, L/16]
        out[f"drel{st}"] = drel.reshape(L // P, P).T.astype(ml_dtypes.bfloat16).copy()
    deg = np.bincount(d, minlength=NT * P).astype(np.int32)
    out["deg"] = deg.reshape(NT, P).T.copy()                       # [128, NT]
    return out


def _host_prep(x, edge_index, W1l, b1l, W1r, W2l, b2l, W2r, Wc, bc):
    src = np.asarray(edge_index[0], np.int64)
    dst = np.asarray(edge_index[1], np.int64)
    x = np.ascontiguousarray(np.asarray(x, np.float32))

    cnts = np.stack([_prep_core(src, dst, m) for m in range(CORES)])  # [C,NT,2]
    budgets = ((cnts.max(axis=0) + P - 1) // P).astype(np.int64)      # [NT,2]

    iota = np.broadcast_to(np.arange(P, dtype=np.float32), (P, P)).copy()
    common = {
        "x_full": x,
        "iota": iota.astype(ml_dtypes.bfloat16),
        "w1lt": np.ascontiguousarray(np.asarray(W1l, np.float32).T),
        "w1rt": np.ascontiguousarray(np.asarray(W1r, np.float32).T),
        "w2lt": np.ascontiguousarray(np.asarray(W2l, np.float32).T),
        "w2rt": np.ascontiguousarray(np.asarray(W2r, np.float32).T),
        "wct": np.ascontiguousarray(np.asarray(Wc, np.float32).T),
        "b1l": np.asarray(b1l, np.float32).reshape(D, 1).copy(),
        "b2l": np.asarray(b2l, np.float32).reshape(D, 1).copy(),
        "bcb": np.tile(np.asarray(bc, np.float32), (P, 1)).copy(),
    }
    in_maps = []
    for m in range(CORES):
        core = _prep_core(src, dst, m, budgets)
        xT = np.zeros((D, NT * P), np.float32)
        xT[:, :NSH] = x[m * NSH:(m + 1) * NSH].T
        core["xt"] = xT
        core.update(common)
        in_maps.append(core)
    return in_maps, budgets


# ------------------------------------------------------------- device build
def _build(nc: bacc.Bacc, budgets):
    bf16 = mybir.dt.bfloat16
    f32 = mybir.dt.float32

    slots = budgets * P
    L_st = [int(slots[:, st].sum()) for st in (0, 1)]
    base = np.zeros((NT, 2), np.int64)
    for st in (0, 1):
        base[:, st] = np.concatenate(([0], np.cumsum(slots[:, st])[:-1]))

    # DRAM parameters
    x_full = nc.declare_dram_parameter("x_full", [N, D], f32, isOutput=False)
    xt = nc.declare_dram_parameter("xt", [D, NT * P], f32, isOutput=False)
    iota = nc.declare_dram_parameter("iota", [P, P], bf16, isOutput=False)
    idx_p = [nc.declare_dram_parameter(f"idx{st}", [P, L_st[st] // 16],
                                       mybir.dt.int16, isOutput=False)
             for st in (0, 1)]
    drel_p = [nc.declare_dram_parameter(f"drel{st}", [P, L_st[st] // P], bf16,
                                        isOutput=False) for st in (0, 1)]
    deg_p = nc.declare_dram_parameter("deg", [P, NT], mybir.dt.int32, isOutput=False)
    w_p = {k: nc.declare_dram_parameter(k, [D, D], f32, isOutput=False)
           for k in ("w1lt", "w1rt", "w2lt", "w2rt")}
    wct_p = nc.declare_dram_parameter("wct", [D, NCLS], f32, isOutput=False)
    b1l_p = nc.declare_dram_parameter("b1l", [D, 1], f32, isOutput=False)
    b2l_p = nc.declare_dram_parameter("b2l", [D, 1], f32, isOutput=False)
    bcb_p = nc.declare_dram_parameter("bcb", [P, NCLS], f32, isOutput=False)
    out_p = nc.declare_dram_parameter("out", [NSH, NCLS], f32, isOutput=True)

    h_local = nc.dram_tensor("h_local", [NSH, D], bf16)
    h_full = nc.dram_tensor("h_full", [N, D], bf16, addr_space="Shared")

    def bcast_mid(ap2d, nb):
        """[128, X] AP -> [128, nb, X] re-reading the X cols nb times."""
        return bass.AP(ap2d.tensor, ap2d.offset, [ap2d.ap[0], [0, nb], list(ap2d.ap[1])])

    def bcast_last(ap2d, n):
        """[128, X] AP -> [128, X, n] broadcasting each element n times."""
        return bass.AP(ap2d.tensor, ap2d.offset, [ap2d.ap[0], list(ap2d.ap[1]), [0, n]])

    with tile.TileContext(nc) as tc:
        with (
            tc.tile_pool(name="cst", bufs=1) as cst,
            tc.tile_pool(name="sb", bufs=2) as sb,
            tc.tile_pool(name="xgp", bufs=4) as xgp,
            tc.tile_pool(name="xbp", bufs=6) as xbp,
            tc.tile_pool(name="sp", bufs=3) as spool,
            tc.tile_pool(name="ps", bufs=2, space="PSUM") as ps,
        ):
            # ---- constants ----
            iota_sb = cst.tile([P, P], bf16)
            nc.scalar.dma_start(out=iota_sb[:, :], in_=iota[:, :])
            ident = cst.tile([P, P], bf16)
            make_identity(nc, ident[:, :])

            wb = {}
            for k in ("w1lt", "w1rt", "w2lt", "w2rt"):
                wf = sb.tile([D, D], f32, tag="wload")
                nc.scalar.dma_start(out=wf[:, :], in_=w_p[k][:, :])
                wb[k] = cst.tile([D, D], bf16, tag=f"w_{k}", name=f"w_{k}")
                nc.vector.tensor_copy(wb[k][:, :], wf[:, :])
            wcf = sb.tile([D, NCLS], f32, tag="wload")
            nc.scalar.dma_start(out=wcf[:, :], in_=wct_p[:, :])
            wcb = cst.tile([D, NCLS], bf16)
            nc.vector.tensor_copy(wcb[:, :], wcf[:, :])

            b1l_sb = cst.tile([D, 1], f32)
            nc.scalar.dma_start(out=b1l_sb[:, :], in_=b1l_p[:, :])
            b2l_sb = cst.tile([D, 1], f32)
            nc.scalar.dma_start(out=b2l_sb[:, :], in_=b2l_p[:, :])
            bcb_sb = cst.tile([P, NCLS], f32)
            nc.scalar.dma_start(out=bcb_sb[:, :], in_=bcb_p[:, :])

            # inverse degree: 1 / max(deg, 1)
            deg_i = sb.tile([P, NT], mybir.dt.int32, tag="degl")
            nc.scalar.dma_start(out=deg_i[:, :], in_=deg_p[:, :])
            deg_f = sb.tile([P, NT], f32, tag="degf")
            nc.vector.tensor_copy(deg_f[:, :], deg_i[:, :])
            nc.vector.tensor_scalar_max(deg_f[:, :], deg_f[:, :], 1.0)
            invdeg = cst.tile([P, NT], f32)
            nc.vector.reciprocal(invdeg[:, :], deg_f[:, :])

            # xT in bf16 (padded feature-major local x)
            xt_f = cst.tile([D, NT * P], f32)
            nc.scalar.dma_start(out=xt_f[:, :], in_=xt[:, :])
            xt_b = cst.tile([D, NT * P], bf16)
            nc.vector.tensor_copy(xt_b[:, :], xt_f[:, :])

            # hT persistent (layer-1 output, feature-major, bf16)
            ht_b = cst.tile([D, NT * P], bf16)

            # gather metadata in SBUF
            idx_sb, drel_sb = [], []
            for st in (0, 1):
                it = cst.tile([P, L_st[st] // 16], mybir.dt.int16, tag=f"idxsb{st}", name=f"idxsb{st}")
                nc.scalar.dma_start(out=it[:, :], in_=idx_p[st][:, :])
                idx_sb.append(it)
                dt_ = cst.tile([P, L_st[st] // P], bf16, tag=f"drelsb{st}", name=f"drelsb{st}")
                nc.scalar.dma_start(out=dt_[:, :], in_=drel_p[st][:, :])
                drel_sb.append(dt_)

            # ------------------------------------------------ one layer
            def do_layer(lay):
                if "nooff" in os.environ.get("GNN_DEBUG", ""):
                    src_half = [x_full[0:HALF, :], x_full[0:HALF, :]]
                elif lay == 0:
                    src_half = [x_full[0:HALF, :], x_full[HALF:N, :]]
                else:
                    src_half = [h_full[0:HALF, :], h_full[HALF:N, :]]
                wl = wb["w1lt"] if lay == 0 else wb["w2lt"]
                wr = wb["w1rt"] if lay == 0 else wb["w2rt"]
                bias = b1l_sb if lay == 0 else b2l_sb
                rhs_loc = xt_b if lay == 0 else ht_b

                chunk_tiles = [{}, {}]  # per stream: chunk id -> bf16 tile
                qrr = [0]  # round-robin swdge queue for gather DGE parallelism

                dbg = os.environ.get("GNN_DEBUG", "")

                def get_chunk(st, c):
                    if c in chunk_tiles[st]:
                        return chunk_tiles[st][c]
                    ln = min(CH, L_st[st] - c * CH)
                    idx_ap = idx_sb[st][:, c * CH // 16:(c * CH + ln) // 16]
                    if "nogather" in dbg:
                        xb = xbp.tile([P, CH // P, D], bf16, tag=f"xb{st}")
                        nc.vector.memset(xb[:, :, :], 1.0)
                    elif lay == 0:
                        xg = xgp.tile([P, CH // P, D], f32, tag=f"xg{st}")
                        nc.gpsimd.dma_gather(
                            out_ap=xg[:, :ln // P, :], in_ap=src_half[st],
                            idxs_ap=idx_ap, num_idxs=ln, num_idxs_reg=ln,
                            elem_size=D, single_packet=SINGLE_PACKET,
                            queue_num=qrr[0])
                        qrr[0] = (qrr[0] + 1) % NQ
                        xb = xbp.tile([P, CH // P, D], bf16, tag=f"xb{st}")
                        nc.scalar.activation(xb[:, :ln // P, :], xg[:, :ln // P, :],
                                             mybir.ActivationFunctionType.Copy)
                    else:
                        xb = xbp.tile([P, CH // P, D], bf16, tag=f"xb{st}")
                        nc.gpsimd.dma_gather(
                            out_ap=xb[:, :ln // P, :], in_ap=src_half[st],
                            idxs_ap=idx_ap, num_idxs=ln, num_idxs_reg=ln,
                            elem_size=D, single_packet=SINGLE_PACKET,
                            queue_num=qrr[0])
                        qrr[0] = (qrr[0] + 1) % NQ
                    chunk_tiles[st][c] = xb
                    return xb

                for t in range(NT):
                    rows = min(P, NSH - t * P)
                    pm = ps.tile([P, D], f32, tag="msg")
                    blocks = []
                    for st in (0, 1):
                        nb = int(budgets[t, st])
                        if nb == 0:
                            continue
                        b0 = int(base[t, st]) // P
                        # S for this (tile, stream): [128, nb, 128] bf16
                        S = spool.tile([P, nb, P], bf16, tag="s")
                        d_ap = drel_sb[st][:, b0:b0 + nb]
                        if "nos" in dbg:
                            nc.vector.memset(S[:, :, :], 0.0)
                        else:
                            nc.vector.tensor_tensor(
                                out=S[:, :, :], in0=bcast_mid(iota_sb[:, :], nb),
                                in1=bcast_last(d_ap, P), op=mybir.AluOpType.is_equal)
                        for b in range(nb):
                            slot = int(base[t, st]) + b * P
                            xb = get_chunk(st, slot // CH)
                            blocks.append((S[:, b, :], xb[:, (slot % CH) // P, :]))
                    for i, (s_ap, x_ap) in enumerate(blocks):
                        nc.tensor.matmul(pm[:, :], lhsT=s_ap, rhs=x_ap,
                                         start=(i == 0), stop=(i == len(blocks) - 1))

                    # mean (node-major, bf16) then transpose to feature-major
                    mean_b = sb.tile([P, D], bf16, tag="mean")
                    nc.vector.tensor_scalar(
                        out=mean_b[:, :], in0=pm[:, :],
                        scalar1=invdeg[:, t:t + 1], scalar2=None,
                        op0=mybir.AluOpType.mult)
                    meanT = sb.tile([P, D], bf16, tag="meanT")
                    if "notr" in dbg:
                        nc.vector.tensor_copy(meanT[:, :], mean_b[:, :])
                    else:
                        pt = ps.tile([P, D], bf16, tag="tr")
                        nc.tensor.transpose(pt[:, :], mean_b[:, :], ident[:, :])
                        nc.scalar.activation(meanT[:, :], pt[:, :],
                                             mybir.ActivationFunctionType.Copy)

                    # hT[o, n] = relu(Wl @ meanT + Wr @ xT + b)
                    ph = ps.tile([D, P], f32, tag="hT")
                    nc.tensor.matmul(ph[:, :], lhsT=wl[:, :], rhs=meanT[:, :],
                                     start=True, stop=False)
                    nc.tensor.matmul(ph[:, :], lhsT=wr[:, :],
                                     rhs=rhs_loc[:, t * P:(t + 1) * P],
                                     start=False, stop=True)

                    if lay == 0:
                        hT = ht_b[:, t * P:(t + 1) * P]
                        if "norelu" in dbg:
                            nc.scalar.activation(hT, ph[:, :],
                                                 mybir.ActivationFunctionType.Copy)
                        else:
                            nc.scalar.activation(hT, ph[:, :],
                                                 mybir.ActivationFunctionType.Relu,
                                                 bias=bias[:, :])
                        h_sb = sb.tile([P, D], bf16, tag="hs")
                        if "notr" in dbg:
                            nc.vector.tensor_copy(h_sb[:, :], hT)
                        else:
                            # node-major copy for the AllGather / layer-2 gather
                            phn = ps.tile([P, D], bf16, tag="aux")
                            nc.tensor.transpose(phn[:, :], hT, ident[:, :])
                            nc.vector.tensor_copy(h_sb[:, :], phn[:, :])
                        nc.scalar.dma_start(out=h_local[t * P:t * P + rows, :],
                                          in_=h_sb[:rows, :])
                    else:
                        embT = sb.tile([D, P], bf16, tag="embT")
                        nc.scalar.activation(embT[:, :], ph[:, :],
                                             mybir.ActivationFunctionType.Relu,
                                             bias=bias[:, :])
                        pc = ps.tile([P, NCLS], f32, tag="aux")
                        nc.tensor.matmul(pc[:, :], lhsT=embT[:, :], rhs=wcb[:, :],
                                         start=True, stop=True)
                        oc = sb.tile([P, NCLS], f32, tag="oc")
                        nc.vector.tensor_tensor(out=oc[:, :], in0=pc[:, :],
                                                in1=bcb_sb[:, :],
                                                op=mybir.AluOpType.add)
                        nc.scalar.dma_start(out=out_p[t * P:t * P + rows, :],
                                          in_=oc[:rows, :])

            dbg = os.environ.get("GNN_DEBUG", "")
            do_layer(0)
            if "nocoll" in dbg:
                nc.gpsimd.dma_start(out=h_full[0:NSH, :], in_=h_local[:, :])
            else:
                nc.gpsimd.collective_compute(
                    "AllGather", mybir.AluOpType.bypass,
                    replica_groups=[list(range(CORES))],
                    ins=[h_local[:, :].opt()], outs=[h_full[:, :].opt()])
            if "nolay2" not in dbg:
                do_layer(1)
            else:
                for t in range(NT):
                    rows = min(P, NSH - t * P)
                    oc = sb.tile([P, NCLS], f32, tag="oc")
                    nc.vector.tensor_copy(oc[:, :], bcb_sb[:, :])
                    nc.scalar.dma_start(out=out_p[t * P:t * P + rows, :],
                                        in_=oc[:rows, :])
    return nc


# ------------------------------------------------------------------- driver
def _enable_axon_trace():
    """The agent image's antenv lacks axon_hooks; synthesize it from the
    ctypes NTFF hook in trn_agent_boot so trace=True works under axon."""
    import sys
    import types
    try:
        import antenv.axon_hooks  # noqa: F401
        return True
    except ImportError:
        pass
    try:
        from trn_agent_boot.trn_boot import _ntff_profile_via_ctypes
        hook = _ntff_profile_via_ctypes("/opt/axon/libaxon_pjrt.so")
        if hook is None:
            return False
        mod = types.ModuleType("antenv.axon_hooks")
        mod.get_axon_ntff_profile_hook = lambda: hook
        mod.set_axon_ntff_profile_hook = lambda h: None
        sys.modules["antenv.axon_hooks"] = mod
        # artifact upload needs bucket access we don't have here
        from concourse import bass_utils as _bu
        _bu.upload_artifacts = lambda tmpdir: f"file://{tmpdir}"
        return True
    except Exception:
        return False


def kernel(x, edge_index, W1l, b1l, W1r, W2l, b2l, W2r, Wc, bc):
    global last_exec_ns
    in_maps, budgets = _host_prep(x, edge_index, W1l, b1l, W1r, W2l, b2l, W2r,
                                  Wc, bc)
    nc = _build(bacc.Bacc(num_swdge_queues=NQ), budgets)
    nc.compile()
    trace = os.environ.get("GNN_TRACE", "0") == "1" and _enable_axon_trace()
    r = run_bass_kernel_spmd(nc, in_maps, core_ids=list(range(CORES)),
                             trace=trace)
    last_exec_ns = r.exec_time_ns
    out = np.concatenate([r.results[m]["out"] for m in range(CORES)], axis=0)
    return out.astype(np.float32)
